# revision 67
# baseline (speedup 1.0000x reference)
"""Trainium2 Bass kernel for AxialSelfAttention2d.

Reference computation (per batch b):
    qkv = W @ x + b            (1x1 conv; W [3E, E], x [E, S, L], E = 512)
    q, k, v split; q *= Dh**-0.5; per head h: q,k,v [Dh=64, S, L]
    col:  scores[s,t|l] = q[:,s,l].k[:,t,l]; softmax over t; out_col = attn @ v
    row:  scores[l,m|s] = q[:,s,l].k[:,s,m]; softmax over m; out_row = attn @ v
    out = out_col + out_row    -> [H*Dh, S, L]

Sharding: 8 cores = 2 batches x 4 head-pairs. Each core computes 2 heads of
one batch end-to-end (no collectives); the host combines core outputs.

Per-core dataflow (matmul operands fp16, fp32 PSUM), software-pipelined so
PE stays dense (clock p-state!) and Act/DVE/DMA overlap:

  Stage A (QKV): x arrives fp16 (host-cast). 1024-column windows, W^T
  stationary, x moving, K=512 via 4 PSUM accumulation steps per group;
  PSUM->SBUF bias copies alternate DVE/Act. q,k -> q2/k2 [128(2h x 64d), S*L].
  v -> per-head augmented (s,l) tiles: v_a0 rows 0..63 = h0 d's with ones
  rows 64..79; v_a1 rows 64..127 = h1 d's with ones rows 48..63 (partition-
  identity writes from PSUM; ones rows come from a DRAM input). x windows
  are prefetched 2 deep on the dedicated SP DMA queue.

  Attention quad (4 indices, both heads): 8 scoresT matmuls (K=64) into a
  2-bank [128, 1024] PSUM tile; ONE Act exp -> et fp16; per head 4 AV
  matmuls N=65 - the transposed ones rows supply the softmax denominator
  as an extra column (h0: col 64 of an 80-wide block at partition base 0;
  h1: col 63 of a 128-wide block - DMA-transpose sources must be aligned
  partition blocks [0,80) / [0,128)). DVE reciprocal + broadcast divide ->
  src[p, i*128+hd]. Per 16-index half: DMA-transpose (SP queue) -> packed
  [hd, i*128+p] and DMA to DRAM fp16. The AV/divide part of each quad is
  deferred one quad so its exp-wait never blocks the PE queue.

  Phase 1 = stage A + row chunks 0..2: each quad is woven between the 3
  stage-A matmul groups of the next chunk so PSUM copy-bank recycling hides
  behind attention work and the PE stream never idles (p-state stays high).
  vt_row half-chunks transpose off v_a as soon as their windows complete.

  Phase 2 = last row chunk + all col chunks (Act-paced): v_a chunks are
  restrided to (l,s) in per-(head, l-half) pieces spread over Pool/DVE and
  DMA-transposed to vt_col; chunk ch+1's pieces/transposes are emitted
  between ch's quads. Row result = [hd, s*128+l'] -> outr; col result =
  [hd, l*128+s] -> outc; the host adds outr + outc^T (fp32).

DMA queue discipline (in-order queues; head-of-line blocking is the main
serialization hazard): x loads + v transposes + output flushes on SP,
exp/copies only on Act, restrides on Pool/DVE engines.
"""

import numpy as np

NUM_HEADS = 8
DIM_HEAD = 64
EMBED = 512
B, S, L = 2, 128, 128
SL = S * L
N_CORES = 8
HPC = 2  # heads per core

GW = 1024         # spatial columns per x window
NG = SL // GW     # 16 windows
CH = 32           # chunk width (s for row, l for col)
NCH = 128 // CH   # 4 chunks
GPC = NG // NCH   # 4 windows per chunk
QPC = CH // 4     # 8 quads per chunk

_CACHE = {}


def build_program(nc, tc):
    import concourse.bass as bass
    import concourse.mybir as mybir

    f16 = mybir.dt.float16
    f32 = mybir.dt.float32
    AF = mybir.ActivationFunctionType
    OP = mybir.AluOpType
    AP = bass.AP

    x_d = nc.dram_tensor("x16", [EMBED, SL], f16, kind="ExternalInput")
    w_d = nc.dram_tensor("wT", [EMBED, 384], f16, kind="ExternalInput")
    b_d = nc.dram_tensor("bvec", [384], f32, kind="ExternalInput")
    ones_d = nc.dram_tensor("ones16", [16, SL], f16, kind="ExternalInput")
    outr_d = nc.dram_tensor("outr", [128, SL], f16, kind="ExternalOutput")
    outc_d = nc.dram_tensor("outc", [128, SL], f16, kind="ExternalOutput")

    # head h data partitions: h0 -> 0..63 (aug rows 64..79, den col 64 of 65)
    #                         h1 -> 64..127 (aug rows 48..63, den col 0 of 65)
    VBASE = (0, 0)      # transpose source partition base (aligned blocks)
    VCNT = (80, 128)    # transpose source partition count per head
    DOFF = (64, 63)     # offset of the denominator col in the transposed block
    DENC = (64, 0)      # denominator column within the 65-wide AV output

    import os
    stage = os.environ.get("AXIAL_DEBUG_STAGE", "full")

    with tc.tile_pool(name="main", bufs=1) as main_pool:
        q2 = main_pool.tile([128, SL], f16, tag="q2")
        k2 = main_pool.tile([128, SL], f16, tag="k2")
        v_a0 = main_pool.tile([80, SL], f16, tag="v_a0")
        v_a1 = main_pool.tile([128, SL], f16, tag="v_a1")
        v_a = (v_a0, v_a1)
        zero_sb = main_pool.tile([128, 1], f32, tag="zero")
        w_sb = main_pool.tile([128, 4, 384], f16, tag="w_sb")
        b_sb = main_pool.tile([128, 3], f32, tag="b_sb")

        nc.vector.memset(zero_sb[:], 0.0)

        wm_t = [None]

        def warm_pe():
            # dummy matmuls spanning the first x-load latency: the PE clock
            # ramps to full p-state (>3us continuous busy) before real QKV
            # work arrives, instead of ramping through windows 0-1
            wm = main_pool.tile([128, 512], f16, tag="warm")
            nc.vector.memset(wm[:], 0.0)
            wm_t[0] = wm
            for i in range(8):
                wps = ps_a.tile([128, 512], f32, tag="acc")
                nc.tensor.matmul(wps[:], wm[:][:, 0:128], wm[:],
                                 start=True, stop=True)

        def load_consts():
            # emitted after the first x load so w doesn't head-of-line
            # block the QKV pipeline start on the SP DMA queue
            nc.sync.dma_start(w_sb[:],
                              w_d.ap().rearrange("(k c) o -> c k o", k=4))
            nc.sync.dma_start(b_sb[:],
                              b_d.ap().rearrange("(m p) -> p m", p=128))

        def load_ones():
            # den source rows; v_a1 rows 0..47 are junk-initialized only so
            # the full-128-partition transposes read defined memory
            nc.sync.dma_start(v_a0[:][64:80, :], ones_d.ap())
            nc.sync.dma_start(v_a1[:][48:64, :], ones_d.ap())
            for r0 in range(0, 48, 16):
                nc.sync.dma_start(v_a1[:][r0:r0 + 16, :], ones_d.ap())

        qv = q2[:].rearrange("p (s l) -> p s l", l=L)
        kv = k2[:].rearrange("p (s l) -> p s l", l=L)

        xt_tiles = {}

        def load_x(g):
            """Prefetch x window g into SBUF (dedicated SP DMA queue)."""
            if g >= NG:
                return
            xt = xpool.tile([128, 4, GW], f16, tag="x")
            nc.sync.dma_start(
                xt[:], x_d.ap()[:, g * GW:(g + 1) * GW]
                    .rearrange("(k c) n -> c k n", k=4))
            xt_tiles[g] = xt

        def stage_a_group(g, grp):
            """One (m, sg) matmul group of window g: 4 accumulating matmuls
            into a ps bank + one PSUM->SBUF bias copy (alternating DVE/Act)."""
            m, sg = grp // (GW // 512), grp % (GW // 512)
            xt = xt_tiles[g]
            if grp == 3 * (GW // 512) - 1:
                del xt_tiles[g]
            ps = ps_a.tile([128, 512], f32, tag="acc")
            for c in range(4):
                nc.tensor.matmul(
                    ps[:], w_sb[:][:, c, m * 128:(m + 1) * 128],
                    xt[:][:, c, sg * 512:(sg + 1) * 512],
                    start=(c == 0), stop=(c == 3))
            off = g * GW + sg * 512
            alt = (2 * g + sg) % 2
            if m < 2:
                dest = (q2, k2)[m]
                if alt == 0:
                    nc.vector.tensor_scalar_add(
                        dest[:][:, off:off + 512], ps[:],
                        b_sb[:][:, m:m + 1])
                else:
                    nc.scalar.activation(
                        dest[:][:, off:off + 512], ps[:], AF.Identity,
                        bias=b_sb[:][:, m:m + 1])
            else:
                for h in range(2):
                    dst = v_a[h][:][h * 64:(h + 1) * 64, off:off + 512]
                    src = ps[:][h * 64:(h + 1) * 64, :]
                    bias = b_sb[:][h * 64:(h + 1) * 64, 2:3]
                    if alt == 0:
                        nc.scalar.activation(dst, src, AF.Identity,
                                             bias=bias)
                    else:
                        nc.vector.tensor_scalar_add(dst, src, bias)

        def stage_a_window(g):
            for grp in range(3 * (GW // 512)):
                stage_a_group(g, grp)

        def vtr_half(ch, half, vts):
            """DMA-transpose v_a (s,l) half-chunk -> the (h, half) vt tile
            (Act DMA queue; SP stays free for x loads). Per-half tiles keep
            the AV dependency on each transpose separate."""
            for h in range(2):
                sv = v_a[h][:][VBASE[h]:VBASE[h] + VCNT[h], :]
                srcv = AP(sv.tensor,
                          sv.offset + (ch * CH + half * 16) * 128,
                          [list(sv.ap[0]), [128, 16], [1, 128]])
                vt = vts[(h, half)][:]
                nc.sync.dma_start(
                    vt.rearrange("p (m i) -> p m i", i=VCNT[h]), srcv,
                    transpose=True)

        def col_build(ch, step, vls_t, vts):
            """Restride v_a (s,l) -> vls (l,s) and transpose -> vt_col for
            chunk ch, spread over steps 0..4 as per-(head, l-half) pieces so
            nothing bunches at the chunk boundary.

            Restride pieces rotate over Pool/DVE/Act/Pool; each l-half
            transposes (Act DMA queue) as soon as its piece lands.
            """
            def piece(h, lh, eng):
                sv = v_a[h][:][VBASE[h]:VBASE[h] + VCNT[h], :]
                vl = vls_t[h][:][VBASE[h]:VBASE[h] + VCNT[h], :]
                src_ls = AP(sv.tensor,
                            sv.offset + ch * CH + lh * 16,
                            [list(sv.ap[0]), [1, 16], [128, 128]])
                dst_ls = AP(vl.tensor, vl.offset + lh * 16 * 128,
                            [list(vl.ap[0]), [128, 16], [1, 128]])
                eng(dst_ls, src_ls)

            def half_t(h, lh):
                vl = vls_t[h][:][VBASE[h]:VBASE[h] + VCNT[h], :]
                srcv = AP(vl.tensor, vl.offset + lh * 16 * 128,
                          [list(vl.ap[0]), [128, 16], [1, 128]])
                vt = vts[(h, lh)][:]
                nc.sync.dma_start(
                    vt.rearrange("p (m i) -> p m i", i=VCNT[h]), srcv,
                    transpose=True)

            if step == 0:
                piece(0, 0, nc.gpsimd.tensor_copy)
            elif step == 1:
                piece(1, 0, nc.vector.tensor_copy)
                half_t(0, 0)
            elif step == 2:
                piece(0, 1, nc.gpsimd.tensor_copy)
                half_t(1, 0)
            elif step == 3:
                piece(1, 1, nc.vector.tensor_copy)
                half_t(0, 1)
            elif step == 4:
                half_t(1, 1)

        pending = []  # deferred finish-parts (one-quad software pipeline)

        def drain_pending(keep=1):
            while len(pending) > keep:
                pending.pop(0)()

        class AttnDir:
            """Per-direction quad emitter, split into a scores part and a
            deferred finish part (AV + normalize + output flush) so the
            finish's exp-wait never blocks the next PE work."""

            def __init__(self, direction, out_d, tag):
                self.direction = direction
                self.out_d = out_d
                self.tag = tag
                if direction == 1:
                    self.qk_slice = lambda t, h, i: \
                        t[h * 64:(h + 1) * 64, i, :]
                else:
                    self.qk_slice = lambda t, h, i: \
                        t[h * 64:(h + 1) * 64, :, i]
                self.src = None

            def start_chunk(self, ch, vts, src_pool, tr_pool, av_ps):
                self.ch = ch
                self.vts = vts
                self.src_pool = src_pool
                self.tr_pool = tr_pool
                self.av_ps = av_ps

            def quad(self, qw):
                ch, vts = self.ch, self.vts
                if qw % 4 == 0:
                    self.src = self.src_pool.tile([128, 16 * 128], f16,
                                                  tag=self.tag, name="src_t")
                src = self.src
                i0 = ch * CH + qw * 4
                sc = sc_ps.tile([128, 1024], f32, tag="sc")
                for h in range(2):
                    for j in range(4):
                        nc.tensor.matmul(
                            sc[:][:, h * 512 + j * 128:h * 512 + (j + 1) * 128],
                            self.qk_slice(kv, h, i0 + j),
                            self.qk_slice(qv, h, i0 + j),
                            start=True, stop=True)
                et = et_pool.tile([128, 1024], f16, tag="et")
                nc.scalar.activation(et[:], sc[:], AF.Exp,
                                     bias=zero_sb[:][:, 0:1])
                pending.append(lambda: self._finish(ch, qw, vts, et, src))

            def _finish(self, ch, qw, vts, et, src):
                for h in range(2):
                    av = self.av_ps.tile([128, 260], f32, tag="av")
                    vth = vts[(h, qw // 4)]
                    for j in range(4):
                        ir = (qw % 4) * 4 + j
                        o0 = ir * VCNT[h] + (0, 63)[h]
                        nc.tensor.matmul(
                            av[:][:, j * 65:(j + 1) * 65],
                            et[:][:, h * 512 + j * 128:h * 512 + (j + 1) * 128],
                            vth[:][:, o0:o0 + 65],
                            start=True, stop=True)
                    den = den_pool.tile([128, 4], f32, tag="den")
                    nc.vector.reciprocal(
                        den[:], AP(av[:].tensor, av[:].offset + DENC[h],
                                   [list(av[:].ap[0]), [65, 4]]))
                    in0 = AP(av[:].tensor, av[:].offset + (1 - DENC[h] // 64),
                             [list(av[:].ap[0]), [65, 4], [1, 64]])
                    in1 = AP(den[:].tensor, den[:].offset,
                             [list(den[:].ap[0]), [1, 4], [0, 64]])
                    o = AP(src[:].tensor,
                           src[:].offset + (qw * 4 % 16) * 128 + h * 64,
                           [list(src[:].ap[0]), [128, 4], [1, 64]])
                    nc.vector.tensor_tensor(o, in0, in1, OP.mult)
                if qw % 4 == 3:
                    half = qw // 4
                    tr = self.tr_pool.tile([128, 16 * 128], f16,
                                           tag="t" + self.tag)
                    nc.sync.dma_start(
                        tr[:].rearrange("p (m i) -> p m i", i=128), src[:],
                        transpose=True)
                    o0 = (ch * CH + half * 16) * 128
                    nc.sync.dma_start(
                        self.out_d.ap()[:, o0:o0 + 16 * 128], tr[:])

        with tc.tile_pool(name="sc_ps", bufs=2, space="PSUM") as sc_ps, \
             tc.tile_pool(name="vt", bufs=2) as vt_pool, \
             tc.tile_pool(name="et", bufs=2) as et_pool, \
             tc.tile_pool(name="denp", bufs=4) as den_pool, \
             tc.tile_pool(name="srcp", bufs=3) as src_pool, \
             tc.tile_pool(name="trp", bufs=2) as tr_pool:
            row = AttnDir(1, outr_d, "sr")

            def new_vts():
                return {(h, hf): vt_pool.tile([128, 16 * 80], f16,
                                              tag=f"vt{h}{hf}",
                                              name=f"vt{h}{hf}_t")
                        for h in range(2) for hf in range(2)}

            # ------- phase 1: stage A + row chunks 0..2 (pipelined) -------
            with tc.tile_pool(name="ps_a", bufs=2, space="PSUM") as ps_a, \
                 tc.tile_pool(name="av_ps1", bufs=2, space="PSUM") as av_ps1, \
                 tc.tile_pool(name="xp", bufs=3) as xpool:

                def new_vts():
                    return {(h, hf): vt_pool.tile([128, 16 * VCNT[h]], f16,
                                                  tag=f"vt{h}{hf}",
                                                  name=f"vt{h}{hf}_t")
                            for h in range(2) for hf in range(2)}

                # prologue: chunk 0 of stage A + its vt_row
                load_x(0)
                load_consts()
                load_x(1)
                warm_pe()
                cur_vts = new_vts()
                for g in range(GPC):
                    load_x(g + 2)
                    if g == 1:
                        load_ones()
                    stage_a_window(g)
                    if g == 1:
                        vtr_half(0, 0, cur_vts)
                if stage == "a":
                    for g in range(NG - GPC):
                        load_x(GPC + g + 2)
                        stage_a_window(GPC + g)
                    for chx in range(NCH):
                        nc.sync.dma_start(
                            outr_d.ap()[:, chx * CH * 128:(chx + 1) * CH * 128],
                            q2[:][:, chx * CH * 128:(chx + 1) * CH * 128])
                        nc.sync.dma_start(
                            outc_d.ap()[:, chx * CH * 128:(chx + 1) * CH * 128],
                            q2[:][:, chx * CH * 128:(chx + 1) * CH * 128])
                    return
                vtr_half(0, 1, cur_vts)

                # weave: per quad, 3 stage-A matmul groups of chunk ch+1 are
                # interleaved with the scores / deferred-finish parts so the
                # PE stream never starves while PSUM copy banks recycle.
                for ch in range(NCH - 1):
                    row.start_chunk(ch, cur_vts, src_pool, tr_pool,
                                    av_ps1)
                    nxt_vts = new_vts()
                    for qw in range(QPC):
                        g = (ch + 1) * GPC + qw // 2
                        gb = (qw % 2) * 3
                        if qw % 2 == 0:
                            load_x(g + 2)
                        stage_a_group(g, gb + 0)
                        row.quad(qw)
                        stage_a_group(g, gb + 1)
                        drain_pending(keep=1)
                        stage_a_group(g, gb + 2)
                        if qw == 3:
                            vtr_half(ch + 1, 0, nxt_vts)
                        elif qw == 7:
                            vtr_half(ch + 1, 1, nxt_vts)
                    cur_vts = nxt_vts
                drain_pending(keep=0)

            if stage == "row":
                row.start_chunk(NCH - 1, cur_vts, None, None, None)
                for chx in range(NCH):
                    nc.sync.dma_start(
                        outc_d.ap()[:, chx * CH * 128:(chx + 1) * CH * 128],
                        q2[:][:, chx * CH * 128:(chx + 1) * CH * 128])
                return

            # ------- phase 2: last row chunk + col, Act-paced merge -------
            with tc.tile_pool(name="vls", bufs=1) as vls_pool, \
                 tc.tile_pool(name="av_ps2", bufs=4, space="PSUM") as av_ps2:
                col = AttnDir(0, outc_d, "sr")

                def new_vls():
                    return [vls_pool.tile([128, CH * 128], f16, tag=f"vls{h}",
                                          name=f"vls{h}_t")
                            for h in range(2)]

                # row chunk 3 quads fill the gap while col chunk 0's vt
                # builds (restrides + transposes)
                row.start_chunk(NCH - 1, cur_vts, src_pool, tr_pool,
                                av_ps2)
                cur_vls = new_vls()
                cvts = new_vts()
                for qw in range(QPC):
                    if qw < 5:
                        col_build(0, qw, cur_vls, cvts)
                    row.quad(qw)
                    drain_pending(keep=1)

                for ch in range(NCH):
                    col.start_chunk(ch, cvts, src_pool, tr_pool, av_ps2)
                    nxt_vls = new_vls() if ch + 1 < NCH else None
                    nxt_vts = new_vts() if ch + 1 < NCH else None
                    for qw in range(QPC):
                        if ch + 1 < NCH and qw < 5:
                            col_build(ch + 1, qw, nxt_vls, nxt_vts)
                        col.quad(qw)
                        drain_pending(keep=1)
                    cur_vls, cvts = nxt_vls, nxt_vts
                drain_pending(keep=0)


def _get_nc():
    if "nc" in _CACHE:
        return _CACHE["nc"]
    import concourse.bacc as bacc
    import concourse.tile as tile

    nc = bacc.Bacc(None, target_bir_lowering=False, debug=False,
                   num_devices=N_CORES)
    with tile.TileContext(nc) as tc:
        build_program(nc, tc)
    nc.compile()
    _CACHE["nc"] = nc
    return nc


def make_in_maps(x, W, b):
    x = np.asarray(x, dtype=np.float32)
    W = np.asarray(W, dtype=np.float32)
    b = np.asarray(b, dtype=np.float32)
    scale = np.float32(DIM_HEAD ** -0.5)
    ones16 = np.ones((16, SL), dtype=np.float16)
    in_maps = []
    for c in range(N_CORES):
        bb, h0 = c // 4, 2 * (c % 4)
        hd = np.arange(h0 * 64, (h0 + 2) * 64)
        sel = np.concatenate([hd, EMBED + hd, 2 * EMBED + hd])
        W_loc = W[sel, :].copy()
        b_loc = b[sel].copy()
        W_loc[:128] *= scale
        b_loc[:128] *= scale
        in_maps.append({
            "x16": x[bb].reshape(EMBED, SL).astype(np.float16),
            "wT": np.ascontiguousarray(W_loc.T).astype(np.float16),
            "bvec": b_loc.astype(np.float32),
            "ones16": ones16,
        })
    return in_maps


def assemble(results):
    out = np.empty((B, EMBED, S, L), dtype=np.float32)
    for c, r in enumerate(results):
        bb, h0 = c // 4, 2 * (c % 4)
        rr = r["outr"].astype(np.float32).reshape(128, S, L)
        cc = r["outc"].astype(np.float32).reshape(128, L, S)
        out[bb, h0 * 64:(h0 + 2) * 64] = rr + cc.transpose(0, 2, 1)
    return out


def kernel(x, W, b):
    from concourse.bass_utils import run_bass_kernel_spmd
    nc = _get_nc()
    res = run_bass_kernel_spmd(nc, make_in_maps(x, W, b),
                               core_ids=list(range(N_CORES)))
    return assemble(res.results)
